# revision 2
# baseline (speedup 1.0000x reference)
"""BinaryLlamaDecoderLayer on 8 TRN2 NeuronCores.

Token-parallel (2 batches x 4 sequence chunks), weights replicated as
NEFF-baked constants (loaded to HBM once at model load). Per-call io is ONE
packed input tensor and ONE bf16 output per core (the axon exec path charges
~2.3ms per io tensor per call, independent of size).

On-chip: weights stored block-row-major so every weight load is a single
contiguous [128, 1024+] slab DMA; q stays in SBUF (no DRAM bounce); attention
probs are transposed on the tensor engine (PE) instead of 2048 DMA
transposes; causal mask and rope tables built/kept on chip. q/k use a 3-term
bf16 hi/lo split for fp32-grade scores (binarized model -> near-one-hot
softmax; score precision decides correctness).
"""
import hashlib
import math
import numpy as np
import ml_dtypes

import concourse.bass as bass
import concourse.bacc as bacc
import concourse.mybir as mybir
from concourse import tile
from concourse.bass_utils import run_bass_kernel_spmd

BF = ml_dtypes.bfloat16
F32, BF16 = mybir.dt.float32, mybir.dt.bfloat16
AF = mybir.ActivationFunctionType
OP = mybir.AluOpType

B, S, H = 2, 2048, 2048
NH, NKV, HD = 32, 8, 64
GR = NH // NKV
FF = 5632
EPS = 1e-5
N_CORES = 8
T = (B * S) // N_CORES        # 512 tokens per core
QT = T // 128                 # 4 query tiles per core
KB = S // 512                 # 4 key blocks of 512
SKT = S // 128                # 16 key tiles of 128
HPT = H // 128                # 16 hidden partition tiles
FFT = FF // 128               # 44 ff tiles
ROPE_BASE = 10000.0

# packed input blob rows: x_t | cos2h | sroth | jrow(4) | qpos(4)
XB = H + 64 + 64 + 4 + 4      # 2184

_CACHE = {}


def _build_nc(shared):
    nc = bacc.Bacc("TRN2", target_bir_lowering=False, debug=False,
                   num_devices=N_CORES)
    blob = nc.dram_tensor("blob", [XB, T], F32, kind="ExternalInput").ap()
    x_t = blob[0:H, :]
    cos2h = blob[H:H + 64, :]
    sroth = blob[H + 64:H + 128, :]
    jrow4 = blob[H + 128:H + 132, :]     # [4, 512] = jrow [1, 2048]
    qpos4 = blob[H + 132:H + 136, :]     # row qt: qpos[:, qt] in cols 0:128

    # weights: block-row-major slabs  D[mt*128+p, kt*128+c] = wT[kt*128+p, mt*128+c]
    qw_hi = nc.inline_tensor(shared["qw_hi"], name="qw_hi").ap()
    qw_lo = nc.inline_tensor(shared["qw_lo"], name="qw_lo").ap()
    kw_hi = nc.inline_tensor(shared["kw_hi"], name="kw_hi").ap()
    kw_lo = nc.inline_tensor(shared["kw_lo"], name="kw_lo").ap()
    vw    = nc.inline_tensor(shared["vw"], name="vw").ap()      # [H, 512]
    ow    = nc.inline_tensor(shared["ow"], name="ow").ap()
    gw    = nc.inline_tensor(shared["gw"], name="gw").ap()
    uw    = nc.inline_tensor(shared["uw"], name="uw").ap()
    dw    = nc.inline_tensor(shared["dw"], name="dw").ap()
    out_d = nc.dram_tensor("out", [H, T], BF16, kind="ExternalOutput").ap()

    with tile.TileContext(nc) as tc:
        with tc.tile_pool(name="const", bufs=1) as cpool, \
             tc.tile_pool(name="bb", bufs=1) as bpool, \
             tc.tile_pool(name="attn", bufs=1) as apool, \
             tc.tile_pool(name="kv", bufs=2) as kvpool, \
             tc.tile_pool(name="work", bufs=2) as wpool, \
             tc.tile_pool(name="pt", bufs=1) as ptpool, \
             tc.tile_pool(name="wt", bufs=2) as wtpool, \
             tc.tile_pool(name="small", bufs=3) as spool, \
             tc.tile_pool(name="psum", bufs=1, space="PSUM") as pspool, \
             tc.tile_pool(name="psumw", bufs=2, space="PSUM") as pwpool, \
             tc.tile_pool(name="psumt", bufs=2, space="PSUM") as ptppool, \
             tc.tile_pool(name="dram", bufs=1, space="DRAM") as dpool:

            ones128 = cpool.tile([128, 1], F32, tag="ones128")
            nc.vector.memset(ones128[:], 1.0)
            ones1 = cpool.tile([1, 128], F32, tag="ones1")
            nc.vector.memset(ones1[:], 1.0)
            cos_t = cpool.tile([128, T], F32, tag="cos2")
            nc.sync.dma_start(cos_t[0:64, :], cos2h[:])
            nc.sync.dma_start(cos_t[64:128, :], cos2h[:])
            srot_t = cpool.tile([128, T], F32, tag="srot")
            nc.sync.dma_start(srot_t[0:64, :], sroth[:])
            nc.sync.dma_start(srot_t[64:128, :], sroth[:])
            jrow_t = cpool.tile([1, S], F32, tag="jrow")
            for k in range(4):
                nc.sync.dma_start(jrow_t[0:1, k * 512:(k + 1) * 512],
                                  jrow4[k:k + 1, :])
            qpos_t = cpool.tile([128, QT], F32, tag="qpos")
            for qt in range(QT):
                nc.sync.dma_start(qpos_t[:, qt:qt + 1],
                                  qpos4[qt:qt + 1, 0:128].rearrange("a b -> b a"))

            ident = cpool.tile([128, 128], BF16, tag="ident")
            nc.vector.memset(ident[:], 1.0)
            nc.gpsimd.affine_select(ident[:], ident[:], pattern=[[-1, 128]],
                                    compare_op=OP.is_equal, fill=0.0,
                                    base=0, channel_multiplier=1)

            eps_t = cpool.tile([1, 1], F32, tag="eps")
            nc.vector.memset(eps_t[:], EPS)

            # ---------- rmsnorm: stats from a DRAM fp32 [H, T] region ----------
            def rmsnorm_bcast(src_dram):
                ssum = pwpool.tile([1, T], F32, tag="psw")
                for pt in range(HPT):
                    xt = wpool.tile([128, T], F32, tag="xin")
                    nc.sync.dma_start(xt[:], src_dram[pt * 128:(pt + 1) * 128, :])
                    sq = wpool.tile([128, T], F32, tag="hf")
                    nc.vector.tensor_tensor(sq[:], xt[:], xt[:], OP.mult)
                    nc.tensor.matmul(ssum[:], ones128[:], sq[:],
                                     start=(pt == 0), stop=(pt == HPT - 1))
                std = spool.tile([1, T], F32, tag="std")
                nc.scalar.activation(std[:], ssum[:], AF.Sqrt, bias=eps_t[:], scale=1.0 / H)
                rstd = spool.tile([1, T], F32, tag="rstd")
                nc.vector.reciprocal(rstd[:], std[:])
                bc = pwpool.tile([128, T], F32, tag="psw")
                nc.tensor.matmul(bc[:], ones1[:], rstd[:], start=True, stop=True)
                bcs = wpool.tile([128, T], F32, tag="bcs", bufs=1)
                nc.vector.tensor_copy(bcs[:], bc[:])
                return bcs

            # ---------- phase 1: rmsnorm1 -> h hi/lo (bb 0..31) ----------
            bb = [bpool.tile([128, T], BF16, tag=f"bb{i}", name=f"bb{i}") for i in range(64)]
            h_hi = bb[0:HPT]
            h_lo = bb[HPT:2 * HPT]
            q_sb = bb[2 * HPT:2 * HPT + NH]     # 32 q tiles (hi rows 0:64, lo 64:128)
            bc1 = rmsnorm_bcast(x_t)
            for pt in range(HPT):
                xt = wpool.tile([128, T], F32, tag="xin")
                nc.sync.dma_start(xt[:], x_t[pt * 128:(pt + 1) * 128, :])
                hf = wpool.tile([128, T], F32, tag="hf")
                nc.vector.tensor_tensor(hf[:], xt[:], bc1[:], OP.mult)
                nc.vector.tensor_copy(h_hi[pt][:], hf[:])
                nc.vector.scalar_tensor_tensor(h_lo[pt][:], hf[:], 1.0, h_hi[pt][:],
                                               OP.mult, OP.subtract)

            # ---------- 3-term projection into psum [128, T]: slab DMAs ----------
            def proj3(ps, w_hi_d, w_lo_d, mt):
                n_mm = 3 * HPT
                i = 0
                for cc in range(2):
                    wh = wtpool.tile([128, 1024], BF16, tag="wh", name="wh")
                    wl = wtpool.tile([128, 1024], BF16, tag="wl", name="wl")
                    nc.sync.dma_start(
                        wh[:], w_hi_d[mt * 128:(mt + 1) * 128,
                                      cc * 1024:(cc + 1) * 1024])
                    nc.sync.dma_start(
                        wl[:], w_lo_d[mt * 128:(mt + 1) * 128,
                                      cc * 1024:(cc + 1) * 1024])
                    for j in range(8):
                        kt = cc * 8 + j
                        for wtile, htile in ((wh, h_hi[kt]), (wh, h_lo[kt]), (wl, h_hi[kt])):
                            nc.tensor.matmul(ps[:], wtile[:, j * 128:(j + 1) * 128],
                                             htile[:], start=(i == 0),
                                             stop=(i == n_mm - 1))
                            i += 1

            # ---------- rope on psum [128, T] (2 heads) ----------
            def rope(ps):
                t1 = wpool.tile([128, T], F32, tag="rope1")
                nc.vector.tensor_tensor(t1[:], ps[:], cos_t[:], OP.mult)
                t2 = wpool.tile([128, T], F32, tag="rope2", bufs=1)
                for g in range(2):
                    o = g * 64
                    nc.vector.tensor_tensor(t2[o:o + 32, :], ps[o + 32:o + 64, :],
                                            srot_t[o:o + 32, :], OP.mult)
                    nc.vector.tensor_tensor(t2[o + 32:o + 64, :], ps[o:o + 32, :],
                                            srot_t[o + 32:o + 64, :], OP.mult)
                nc.vector.tensor_tensor(t1[:], t1[:], t2[:], OP.add)
                return t1

            # ---------- phase 2a: q proj + rope -> q_sb (SBUF resident) ----------
            for mt in range(HPT):        # 2 heads per mt
                ps = pwpool.tile([128, T], F32, tag="psw")
                proj3(ps, qw_hi, qw_lo, mt)
                qr = rope(ps)
                qhi = wpool.tile([128, T], BF16, tag="qhi", bufs=1)
                nc.vector.tensor_copy(qhi[:], qr[:])
                qlo = wpool.tile([128, T], BF16, tag="qlo", bufs=1)
                nc.vector.scalar_tensor_tensor(qlo[:], qr[:], 1.0, qhi[:],
                                               OP.mult, OP.subtract)
                for g in range(2):
                    o = g * 64
                    qt_tile = q_sb[2 * mt + g]
                    nc.gpsimd.tensor_copy(qt_tile[0:64, :], qhi[o:o + 64, :])
                    nc.gpsimd.tensor_copy(qt_tile[64:128, :], qlo[o:o + 64, :])

            # ---------- phase 2b: k proj + rope + split (own tokens) ----------
            k_hi_own, k_lo_own = [], []
            for mt in range(NKV * HD // 128):   # 4 tiles
                ps = pwpool.tile([128, T], F32, tag="psw")
                proj3(ps, kw_hi, kw_lo, mt)
                kr = rope(ps)
                khi = wpool.tile([128, T], BF16, tag=f"khi{mt}", bufs=1)
                nc.vector.tensor_copy(khi[:], kr[:])
                klo = wpool.tile([128, T], BF16, tag=f"klo{mt}", bufs=1)
                nc.vector.scalar_tensor_tensor(klo[:], kr[:], 1.0, khi[:],
                                               OP.mult, OP.subtract)
                k_hi_own.append(khi)
                k_lo_own.append(klo)

            # ---------- phase 2c: v projection (token-major, bf16) ----------
            v_own = []
            for tmt in range(QT):   # 4 token tiles
                ps = pwpool.tile([128, NKV * HD], F32, tag="psw")
                for kt in range(HPT):
                    wv = wtpool.tile([128, NKV * HD], BF16, tag="wv")
                    nc.sync.dma_start(wv[:], vw[kt * 128:(kt + 1) * 128, :])
                    nc.tensor.matmul(ps[:], h_hi[kt][:, tmt * 128:(tmt + 1) * 128],
                                     wv[:], start=(kt == 0), stop=(kt == HPT - 1))
                vt = wpool.tile([128, NKV * HD], BF16, tag=f"vown{tmt}", bufs=1)
                nc.vector.tensor_copy(vt[:], ps[:])
                v_own.append(vt)

            # ---------- phase 3: AllGather k_hi/k_lo/v ----------
            RPR = 1536  # bf16 rows per rank: khi 512, klo 512, v 512
            bounce_in = dpool.tile([RPR, 256], F32, tag="agin")
            bounce_out = dpool.tile([4 * RPR, 256], F32, tag="agout")
            bi_bf = bounce_in.bitcast(BF16)    # [1536, 512] bf16 view
            for mt in range(4):
                nc.sync.dma_start(bi_bf[mt * 128:(mt + 1) * 128, :], k_hi_own[mt][:])
                nc.sync.dma_start(bi_bf[512 + mt * 128:512 + (mt + 1) * 128, :],
                                  k_lo_own[mt][:])
                nc.sync.dma_start(bi_bf[1024 + mt * 128:1024 + (mt + 1) * 128, :],
                                  v_own[mt][:])
            nc.gpsimd.collective_compute(
                "AllGather", OP.bypass,
                replica_groups=[[0, 1, 2, 3], [4, 5, 6, 7]],
                ins=[bounce_in.opt()],
                outs=[bounce_out.opt()],
            )
            bo_bf = bounce_out.bitcast(BF16)   # [6144, 512] bf16 view

            # causal mask on-device: mask[p, j] = 0 if qpos[p,qt] >= j else -1e9
            jbc = pspool.tile([128, S], F32, tag="pss")
            for kb in range(KB):
                sl = slice(kb * 512, (kb + 1) * 512)
                nc.tensor.matmul(jbc[:, sl], ones1[:], jrow_t[:, sl],
                                 start=True, stop=True)
            mask_sb = []
            for qt in range(QT):
                mk = apool.tile([128, S], BF16, tag=f"mask{qt}")
                nc.vector.tensor_scalar(mk[:], jbc[:], qpos_t[:, qt:qt + 1], 0.0,
                                        OP.subtract, OP.is_le)
                nc.scalar.activation(mk[:], mk[:], AF.Copy, bias=-1e9, scale=1e9)
                mask_sb.append(mk)

            attn = []    # 16 tiles [128, T] bf16: attn^T rows = head dims
            for mt in range(HPT):
                attn.append(apool.tile([128, T], BF16, tag=f"attn{mt}", name=f"attn{mt}"))

            # ---------- phase 5: attention ----------
            for hd_ in range(NH):
                kvh = hd_ // GR
                if hd_ % GR == 0:
                    # stream this kv-head's k into SBUF: dup'd hi + lo
                    kd = kvpool.tile([128, S], BF16, tag="kdup", bufs=1)
                    kl = kvpool.tile([64, S], BF16, tag="klo", bufs=1)
                    for r in range(KB):
                        src_hi = bo_bf[r * RPR + kvh * 64: r * RPR + kvh * 64 + 64, :]
                        src_lo = bo_bf[r * RPR + 512 + kvh * 64:
                                       r * RPR + 512 + kvh * 64 + 64, :]
                        nc.sync.dma_start(kd[0:64, r * 512:(r + 1) * 512], src_hi)
                        nc.sync.dma_start(kd[64:128, r * 512:(r + 1) * 512], src_hi)
                        nc.sync.dma_start(kl[:, r * 512:(r + 1) * 512], src_lo)
                    v_kv = []
                    for kt in range(SKT):
                        r, o = kt // 4, kt % 4
                        vt = kvpool.tile([128, HD], BF16, tag=f"vk{kt}", name=f"vk{kt}")
                        nc.sync.dma_start(
                            vt[:],
                            bo_bf[r * RPR + 1024 + o * 128: r * RPR + 1024 + (o + 1) * 128,
                                  kvh * 64:(kvh + 1) * 64])
                        v_kv.append(vt)
                pt_tiles = [ptpool.tile([128, T], BF16, tag=f"pt{kt}", name=f"pt{kt}")
                            for kt in range(SKT)]
                for qt in range(QT):
                    qs = q_sb[hd_][:, qt * 128:(qt + 1) * 128]
                    ps = pspool.tile([128, S], F32, tag="pss")
                    for kb in range(KB):
                        sl = slice(kb * 512, (kb + 1) * 512)
                        nc.tensor.matmul(ps[:, sl], qs, kd[:, sl],
                                         start=True, stop=False)
                        nc.tensor.matmul(ps[:, sl], qs[0:64, :], kl[:, sl],
                                         start=False, stop=True)
                    # in-place mask add on PSUM, then row max, exp, normalize
                    nc.vector.scalar_tensor_tensor(ps[:], ps[:], 1.0, mask_sb[qt][:],
                                                   OP.mult, OP.add)
                    mx = spool.tile([128, 1], F32, tag="mx")
                    nc.vector.tensor_reduce(mx[:], ps[:], axis=mybir.AxisListType.X,
                                            op=OP.max)
                    nmx = spool.tile([128, 1], F32, tag="nmx")
                    nc.vector.tensor_scalar_mul(nmx[:], mx[:], -1.0)
                    pbf = wpool.tile([128, S], BF16, tag="pbf")
                    sume = spool.tile([128, 1], F32, tag="sume")
                    nc.scalar.activation(pbf[:], ps[:], AF.Exp, bias=nmx[:],
                                         scale=1.0, accum_out=sume[:])
                    rsum = spool.tile([128, 1], F32, tag="rsum")
                    nc.vector.reciprocal(rsum[:], sume[:])
                    nc.vector.tensor_scalar_mul(pbf[:], pbf[:], rsum[:])
                    # transpose probs on the tensor engine (PE), copy on Pool
                    # transpose probs via identity matmul (PE): out = pbf_slice^T
                    for kt in range(SKT):
                        tp = ptppool.tile([128, 128], F32, tag="tp")
                        nc.tensor.matmul(tp[:], pbf[:, kt * 128:(kt + 1) * 128],
                                         ident[:], start=True, stop=True)
                        nc.scalar.activation(
                            pt_tiles[kt][:, qt * 128:(qt + 1) * 128], tp[:],
                            AF.Copy)
                pav = pwpool.tile([64, T], F32, tag="psw")
                for kt in range(SKT):
                    nc.tensor.matmul(pav[:], v_kv[kt][:],
                                     pt_tiles[kt][:], start=(kt == 0),
                                     stop=(kt == SKT - 1))
                o = (hd_ % 2) * 64
                nc.vector.tensor_copy(attn[hd_ // 2][o:o + 64, :], pav[:])

            # ---------- phase 6: o-proj + residual -> hid in DRAM ----------
            hid_d = dpool.tile([H, T], F32, tag="hid")
            for mt in range(HPT):
                ps = pwpool.tile([128, T], F32, tag="psw")
                for cc in range(2):
                    wo = wtpool.tile([128, 1024], BF16, tag="wh", name="wo")
                    nc.sync.dma_start(
                        wo[:], ow[mt * 128:(mt + 1) * 128,
                                  cc * 1024:(cc + 1) * 1024])
                    for j in range(8):
                        kt = cc * 8 + j
                        nc.tensor.matmul(ps[:], wo[:, j * 128:(j + 1) * 128],
                                         attn[kt][:], start=(kt == 0),
                                         stop=(kt == HPT - 1))
                xt = wpool.tile([128, T], F32, tag="xin")
                nc.sync.dma_start(xt[:], x_t[mt * 128:(mt + 1) * 128, :])
                ht = wpool.tile([128, T], F32, tag="hf")
                nc.vector.tensor_tensor(ht[:], ps[:], xt[:], OP.add)
                nc.sync.dma_start(hid_d[mt * 128:(mt + 1) * 128, :], ht[:])

            # ---------- phase 7: rmsnorm2 -> h2 (bb 0..15) ----------
            h2 = bb[0:HPT]
            bc2 = rmsnorm_bcast(hid_d)
            for pt in range(HPT):
                xt = wpool.tile([128, T], F32, tag="xin")
                nc.sync.dma_start(xt[:], hid_d[pt * 128:(pt + 1) * 128, :])
                hf = wpool.tile([128, T], F32, tag="hf")
                nc.vector.tensor_tensor(hf[:], xt[:], bc2[:], OP.mult)
                nc.vector.tensor_copy(h2[pt][:], hf[:])

            # ---------- phase 8: gate/up + silu -> act (bb 16..59) ----------
            act = bb[HPT:HPT + FFT]
            for ft in range(FFT):
                psg = pwpool.tile([128, T], F32, tag="psw")
                psu = pwpool.tile([128, T], F32, tag="psw")
                for cc in range(2):
                    wg = wtpool.tile([128, 1024], BF16, tag="wh", name="wg")
                    wu = wtpool.tile([128, 1024], BF16, tag="wl", name="wu")
                    nc.sync.dma_start(
                        wg[:], gw[ft * 128:(ft + 1) * 128,
                                  cc * 1024:(cc + 1) * 1024])
                    nc.sync.dma_start(
                        wu[:], uw[ft * 128:(ft + 1) * 128,
                                  cc * 1024:(cc + 1) * 1024])
                    for j in range(8):
                        kt = cc * 8 + j
                        nc.tensor.matmul(psg[:], wg[:, j * 128:(j + 1) * 128],
                                         h2[kt][:], start=(kt == 0), stop=(kt == HPT - 1))
                        nc.tensor.matmul(psu[:], wu[:, j * 128:(j + 1) * 128],
                                         h2[kt][:], start=(kt == 0), stop=(kt == HPT - 1))
                gs = wpool.tile([128, T], BF16, tag="gs")
                nc.scalar.activation(gs[:], psg[:], AF.Silu)
                nc.vector.tensor_tensor(act[ft][:], gs[:], psu[:], OP.mult)

            # ---------- phase 9: down + residual -> out ----------
            for mt in range(HPT):
                ps = pwpool.tile([128, T], F32, tag="psw")
                for kc in range(4):          # 11 kt per chunk
                    wd = wtpool.tile([128, 128 * 11], BF16, tag="wd")
                    nc.sync.dma_start(
                        wd[:], dw[mt * 128:(mt + 1) * 128,
                                  kc * 1408:(kc + 1) * 1408])
                    for j in range(11):
                        kt = kc * 11 + j
                        nc.tensor.matmul(ps[:], wd[:, j * 128:(j + 1) * 128],
                                         act[kt][:], start=(kt == 0),
                                         stop=(kt == FFT - 1))
                xt = wpool.tile([128, T], F32, tag="xin")
                nc.sync.dma_start(xt[:], hid_d[mt * 128:(mt + 1) * 128, :])
                ot = wpool.tile([128, T], BF16, tag="obf")
                nc.vector.tensor_tensor(ot[:], ps[:], xt[:], OP.add)
                nc.sync.dma_start(out_d[mt * 128:(mt + 1) * 128, :], ot[:])

    nc.compile()
    return nc


def _preprocess(inputs):
    kk = np.float32(inputs["kk"])
    aa = np.float32(inputs["aa"])
    def binw(w):
        return (aa * np.clip(kk * np.asarray(w, dtype=np.float32), -1.0, 1.0))
    ln1 = np.asarray(inputs["ln1_w"], dtype=np.float32)
    ln2 = np.asarray(inputs["ln2_w"], dtype=np.float32)
    qw = binw(inputs["q_w"]) * ln1[None, :] / np.float32(math.sqrt(HD))
    kw = binw(inputs["k_w"]) * ln1[None, :]
    vw = binw(inputs["v_w"]) * ln1[None, :]
    ow = binw(inputs["o_w"])
    gw = binw(inputs["gate_w"]) * ln2[None, :]
    uw = binw(inputs["up_w"]) * ln2[None, :]
    dw = binw(inputs["down_w"])

    def split(w):
        hi = w.astype(BF)
        lo = (w - hi.astype(np.float32)).astype(BF)
        return np.ascontiguousarray(hi), np.ascontiguousarray(lo)

    def slab_major(wt):
        # wt = w^T [K, M] -> D [M, K], D[mt*128+p, kt*128+c] = wt[kt*128+p, mt*128+c]
        K, M = wt.shape
        w4 = wt.reshape(K // 128, 128, M // 128, 128).transpose(2, 1, 0, 3)
        return np.ascontiguousarray(w4.reshape(M, K))

    qw_hi, qw_lo = split(qw.T)     # [H, H]
    kw_hi, kw_lo = split(kw.T)     # [H, 512]
    shared = {
        "qw_hi": slab_major(qw_hi), "qw_lo": slab_major(qw_lo),
        "kw_hi": slab_major(kw_hi), "kw_lo": slab_major(kw_lo),
        "vw": np.ascontiguousarray(vw.T.astype(BF)),
        "ow": slab_major(ow.T.astype(BF)),
        "gw": slab_major(gw.T.astype(BF)),
        "uw": slab_major(uw.T.astype(BF)),
        "dw": slab_major(dw.T.astype(BF)),
    }

    x = np.asarray(inputs["hidden_states"], dtype=np.float32)
    pos = np.asarray(inputs["position_ids"], dtype=np.int32)

    in_maps = []
    for c in range(N_CORES):
        b, ch = c // 4, c % 4
        sl = slice(ch * T, (ch + 1) * T)
        inv = (1.0 / (ROPE_BASE ** (np.arange(0, HD, 2, dtype=np.float32) / np.float32(HD))))
        fr = pos[b, sl].astype(np.float32)[:, None] * inv[None, :]   # [T, 32]
        emb = np.concatenate([fr, fr], axis=-1)                      # [T, 64]
        cos = np.cos(emb).astype(np.float32).T                       # [64, T]
        sin = np.sin(emb).astype(np.float32).T                       # [64, T]
        srot = np.concatenate([-sin[0:32], sin[32:64]], axis=0)      # [64, T]
        jrow4 = pos[b].astype(np.float32).reshape(4, 512)
        qpos4 = np.zeros((4, T), dtype=np.float32)
        qpos4[:, 0:128] = pos[b, sl].astype(np.float32).reshape(QT, 128)
        blob = np.concatenate(
            [x[b, sl].T, cos, srot, jrow4, qpos4], axis=0).astype(np.float32)
        in_maps.append({"blob": np.ascontiguousarray(blob)})
    return shared, in_maps


def kernel(**inputs):
    shared, in_maps = _preprocess(inputs)
    sig = hashlib.sha1()
    for k in sorted(shared):
        sig.update(shared[k].tobytes())
    sig = sig.hexdigest()
    if _CACHE.get("sig") != sig:
        _CACHE["nc"] = _build_nc(shared)
        _CACHE["sig"] = sig
    nc = _CACHE["nc"]
    res = run_bass_kernel_spmd(nc, in_maps, core_ids=list(range(N_CORES)))
    out = np.empty((B, S, H), dtype=np.float32)
    for c in range(N_CORES):
        b, ch = c // 4, c % 4
        out[b, ch * T:(ch + 1) * T, :] = res.results[c]["out"].astype(np.float32).T
    return out
